# revision 1
# baseline (speedup 1.0000x reference)
"""CoPEGate Trainium2 kernel.

Computes out[b,h,t,s] = sigmoid((Q K^T)[b,h,t,s] / sqrt(D)) * (P P^T)[t,s] / sqrt(D)
for B=2, H=12, T=2048, D=64 (fp32), distributed over 8 NeuronCores.

Sharding: the 24 (b,h) pairs are split 3-per-core (head-parallel); the
positional matrix P is replicated and its T x T bias is computed on every
core (it is reused across that core's 3 heads). No cross-device
communication is needed.

Per-core dataflow (all shapes per core):
  inputs   qT, kT: [3, 64, 2048] fp32 (host pre-transposed so the matmul
           contraction dim D=64 lands on SBUF partitions), pT: [64, 2048]
  loop over 16 row-tiles of 128:
    pos stripe   = PE matmul pT[:, tile].T @ pT          -> PSUM [128, 2048]
                   ScalarE Copy * 1/sqrt(D)              -> SBUF
    per head h:  scores = PE matmul qT[h][:, tile].T @ kT[h] -> PSUM
                 gate   = ScalarE Sigmoid(scores / sqrt(D))  -> SBUF
                 out    = VectorE gate * pos stripe          -> SBUF
                 DMA out stripe (1 MiB, contiguous)          -> HBM
Matmuls run in float32r (full-rate fp32, ~1.5e-4 rel err); everything
else is fp32. The kernel is output-DMA bound (~50 MiB written per core).
"""

import math
import os
import sys

import numpy as np

sys.path.insert(0, "/opt/trn_rl_repo")

B, H, T, D = 2, 12, 2048, 64
N_CORES = 8
HPC = (B * H) // N_CORES  # heads per core
PT = 128  # output row-tile height (SBUF/PSUM partitions)
NT = T // PT  # row tiles
NCHUNK = 512  # matmul moving-operand free dim (one PSUM bank of fp32)
NCH = T // NCHUNK
INV_SQRT_D = 1.0 / math.sqrt(D)

_NC_CACHE = {}


def _build_nc():
    import concourse.bass as bass
    from concourse import bacc, mybir, tile

    f32 = mybir.dt.float32
    f32r = mybir.dt.float32r
    Sigmoid = mybir.ActivationFunctionType.Sigmoid
    Copy = mybir.ActivationFunctionType.Copy

    nc = bacc.Bacc("TRN2", target_bir_lowering=False)

    qT = nc.dram_tensor("qT", [HPC, D, T], f32r, kind="ExternalInput")
    kT = nc.dram_tensor("kT", [HPC, D, T], f32r, kind="ExternalInput")
    pT = nc.dram_tensor("pT", [D, T], f32r, kind="ExternalInput")
    out = nc.dram_tensor("out", [HPC, T, T], f32, kind="ExternalOutput")

    with tile.TileContext(nc) as tc:
        with tc.tile_pool(name="ins", bufs=1) as ins_pool, \
             tc.tile_pool(name="pos", bufs=2) as pos_pool, \
             tc.tile_pool(name="gate", bufs=3) as gate_pool, \
             tc.tile_pool(name="outs", bufs=4) as outs_pool, \
             tc.tile_pool(name="ps", bufs=2, space="PSUM") as ps_pool:

            q_sb = []
            k_sb = []
            for h in range(HPC):
                qh = ins_pool.tile([D, T], f32r, tag=f"q{h}")
                nc.sync.dma_start(out=qh, in_=qT[h])
                q_sb.append(qh)
                kh = ins_pool.tile([D, T], f32r, tag=f"k{h}")
                nc.sync.dma_start(out=kh, in_=kT[h])
                k_sb.append(kh)
            p_sb = ins_pool.tile([D, T], f32r, tag="p")
            nc.sync.dma_start(out=p_sb, in_=pT[:])

            for it in range(NT):
                tsl = bass.ts(it, PT)

                pp = ps_pool.tile([PT, T], f32, tag="ps")
                for j in range(NCH):
                    nc.tensor.matmul(
                        pp[:, bass.ts(j, NCHUNK)],
                        p_sb[:, tsl],
                        p_sb[:, bass.ts(j, NCHUNK)],
                        start=True,
                        stop=True,
                    )
                pos_sb = pos_pool.tile([PT, T], f32, tag="pos")
                nc.scalar.activation(pos_sb, pp, Copy, scale=INV_SQRT_D)

                for h in range(HPC):
                    sp = ps_pool.tile([PT, T], f32, tag="ps")
                    for j in range(NCH):
                        nc.tensor.matmul(
                            sp[:, bass.ts(j, NCHUNK)],
                            q_sb[h][:, tsl],
                            k_sb[h][:, bass.ts(j, NCHUNK)],
                            start=True,
                            stop=True,
                        )
                    gate = gate_pool.tile([PT, T], f32, tag="gate")
                    nc.scalar.activation(gate, sp, Sigmoid, scale=INV_SQRT_D)
                    o = outs_pool.tile([PT, T], f32, tag="o")
                    nc.vector.tensor_mul(o, gate, pos_sb)
                    nc.sync.dma_start(out=out[h, tsl, :], in_=o)

    nc.finalize()
    return nc


def _get_nc():
    if "nc" not in _NC_CACHE:
        _NC_CACHE["nc"] = _build_nc()
    return _NC_CACHE["nc"]


def kernel(query, key, pos_embed_weight):
    query = np.ascontiguousarray(np.asarray(query, dtype=np.float32))
    key = np.ascontiguousarray(np.asarray(key, dtype=np.float32))
    pos_embed_weight = np.asarray(pos_embed_weight, dtype=np.float32)

    q = query.reshape(B * H, T, D)
    k = key.reshape(B * H, T, D)
    p_t = np.ascontiguousarray(pos_embed_weight[:T].T)  # [D, T]

    in_maps = []
    for c in range(N_CORES):
        hs = slice(c * HPC, (c + 1) * HPC)
        in_maps.append(
            {
                "qT": np.ascontiguousarray(q[hs].transpose(0, 2, 1)),
                "kT": np.ascontiguousarray(k[hs].transpose(0, 2, 1)),
                "pT": p_t,
            }
        )

    from concourse.bass_utils import run_bass_kernel_spmd

    nc = _get_nc()
    res = run_bass_kernel_spmd(
        nc,
        in_maps,
        core_ids=list(range(N_CORES)),
        trace=bool(os.environ.get("KERNEL_TRACE")),
    )
    kernel.last_results = res

    full = np.empty((B * H, T, T), dtype=np.float32)
    for c in range(N_CORES):
        full[c * HPC : (c + 1) * HPC] = res.results[c]["out"]
    return full.reshape(B, H, T, T)


kernel.last_results = None


# revision 3
# speedup vs baseline: 1.0206x; 1.0206x over previous
"""CoPEGate Trainium2 kernel.

Computes out[b,h,t,s] = sigmoid((Q K^T)[b,h,t,s] / sqrt(D)) * (P P^T)[t,s] / sqrt(D)
for B=2, H=12, T=2048, D=64 (fp32), distributed over 8 NeuronCores.

Sharding: the 24 (b,h) pairs are split 3-per-core (head-parallel); the
positional matrix P is replicated and its T x T bias is computed on every
core (it is reused across that core's 3 heads). No cross-device
communication is needed.

Per-core dataflow (all shapes per core):
  inputs   qT, kT: [3, 64, 2048] fp32 (host pre-transposed so the matmul
           contraction dim D=64 lands on SBUF partitions), pT: [64, 2048]
  loop over 16 row-tiles of 128:
    pos stripe   = PE matmul pT[:, tile].T @ pT          -> PSUM [128, 2048]
                   ScalarE Copy * 1/sqrt(D)              -> SBUF
    per head h:  scores = PE matmul qT[h][:, tile].T @ kT[h] -> PSUM
                 gate   = ScalarE Sigmoid(scores / sqrt(D))  -> SBUF
                 out    = VectorE gate * pos stripe          -> SBUF
                 DMA out stripe (1 MiB, contiguous)          -> HBM
Matmuls run in float32r (full-rate fp32, ~1.5e-4 rel err); everything
else is fp32. The kernel is output-DMA bound (~50 MiB written per core).
"""

import math
import os
import sys

import numpy as np

sys.path.insert(0, "/opt/trn_rl_repo")

B, H, T, D = 2, 12, 2048, 64
N_CORES = 8
HPC = (B * H) // N_CORES  # heads per core
PT = 128  # output row-tile height (SBUF/PSUM partitions)
NT = T // PT  # row tiles
NCHUNK = 512  # matmul moving-operand free dim (one PSUM bank of fp32)
NCH = T // NCHUNK
INV_SQRT_D = 1.0 / math.sqrt(D)

_NC_CACHE = {}


def _build_nc():
    import concourse.bass as bass
    from concourse import bacc, mybir, tile

    f32 = mybir.dt.float32
    f32r = mybir.dt.float32r
    Sigmoid = mybir.ActivationFunctionType.Sigmoid
    Copy = mybir.ActivationFunctionType.Copy

    nc = bacc.Bacc("TRN2", target_bir_lowering=False)

    qT = nc.dram_tensor("qT", [HPC, D, T], f32r, kind="ExternalInput")
    kT = nc.dram_tensor("kT", [HPC, D, T], f32r, kind="ExternalInput")
    pT = nc.dram_tensor("pT", [D, T], f32r, kind="ExternalInput")
    out = nc.dram_tensor("out", [HPC, T, T], f32, kind="ExternalOutput")

    with tile.TileContext(nc) as tc:
        with tc.tile_pool(name="ins", bufs=1) as ins_pool, \
             tc.tile_pool(name="pos", bufs=2) as pos_pool, \
             tc.tile_pool(name="gate", bufs=4) as gate_pool, \
             tc.tile_pool(name="outs", bufs=6) as outs_pool, \
             tc.tile_pool(name="ps", bufs=2, space="PSUM") as ps_pool:

            # Load order matters for pipeline ramp-up: the first stripes
            # need pT (pos bias) and head 0's q/k, so load those first.
            p_sb = ins_pool.tile([D, T], f32r, tag="p")
            nc.sync.dma_start(out=p_sb, in_=pT[:])
            q_sb = []
            k_sb = []
            for h in range(HPC):
                qh = ins_pool.tile([D, T], f32r, tag=f"q{h}")
                nc.sync.dma_start(out=qh, in_=qT[h])
                q_sb.append(qh)
                kh = ins_pool.tile([D, T], f32r, tag=f"k{h}")
                nc.sync.dma_start(out=kh, in_=kT[h])
                k_sb.append(kh)

            for it in range(NT):
                tsl = bass.ts(it, PT)

                pp = ps_pool.tile([PT, T], f32, tag="ps")
                for j in range(NCH):
                    nc.tensor.matmul(
                        pp[:, bass.ts(j, NCHUNK)],
                        p_sb[:, tsl],
                        p_sb[:, bass.ts(j, NCHUNK)],
                        start=True,
                        stop=True,
                    )
                pos_sb = pos_pool.tile([PT, T], f32, tag="pos")
                nc.scalar.activation(pos_sb, pp, Copy, scale=INV_SQRT_D)

                for h in range(HPC):
                    sp = ps_pool.tile([PT, T], f32, tag="ps")
                    for j in range(NCH):
                        nc.tensor.matmul(
                            sp[:, bass.ts(j, NCHUNK)],
                            q_sb[h][:, tsl],
                            k_sb[h][:, bass.ts(j, NCHUNK)],
                            start=True,
                            stop=True,
                        )
                    gate = gate_pool.tile([PT, T], f32, tag="gate")
                    nc.scalar.activation(gate, sp, Sigmoid, scale=INV_SQRT_D)
                    o = outs_pool.tile([PT, T], f32, tag="o")
                    nc.vector.tensor_mul(o, gate, pos_sb)
                    nc.sync.dma_start(out=out[h, tsl, :], in_=o)

    nc.finalize()
    return nc


def _get_nc():
    if "nc" not in _NC_CACHE:
        _NC_CACHE["nc"] = _build_nc()
    return _NC_CACHE["nc"]


def kernel(query, key, pos_embed_weight):
    query = np.ascontiguousarray(np.asarray(query, dtype=np.float32))
    key = np.ascontiguousarray(np.asarray(key, dtype=np.float32))
    pos_embed_weight = np.asarray(pos_embed_weight, dtype=np.float32)

    q = query.reshape(B * H, T, D)
    k = key.reshape(B * H, T, D)
    p_t = np.ascontiguousarray(pos_embed_weight[:T].T)  # [D, T]

    in_maps = []
    for c in range(N_CORES):
        hs = slice(c * HPC, (c + 1) * HPC)
        in_maps.append(
            {
                "qT": np.ascontiguousarray(q[hs].transpose(0, 2, 1)),
                "kT": np.ascontiguousarray(k[hs].transpose(0, 2, 1)),
                "pT": p_t,
            }
        )

    from concourse.bass_utils import run_bass_kernel_spmd

    nc = _get_nc()
    res = run_bass_kernel_spmd(
        nc,
        in_maps,
        core_ids=list(range(N_CORES)),
        trace=bool(os.environ.get("KERNEL_TRACE")),
    )
    kernel.last_results = res

    full = np.empty((B * H, T, T), dtype=np.float32)
    for c in range(N_CORES):
        full[c * HPC : (c + 1) * HPC] = res.results[c]["out"]
    return full.reshape(B, H, T, T)


kernel.last_results = None


# revision 4
# speedup vs baseline: 1.0659x; 1.0443x over previous
"""CoPEGate Trainium2 kernel.

Computes out[b,h,t,s] = sigmoid((Q K^T)[b,h,t,s] / sqrt(D)) * (P P^T)[t,s] / sqrt(D)
for B=2, H=12, T=2048, D=64 (fp32), distributed over 8 NeuronCores.

Sharding: the 24 (b,h) pairs are split 3-per-core (head-parallel); the
positional matrix P is replicated and its T x T bias is computed on every
core (it is reused across that core's 3 heads). No cross-device
communication is needed.

Per-core dataflow (all shapes per core):
  inputs   qT, kT: [3, 64, 2048] bf16 (host pre-transposed so the matmul
           contraction dim D=64 lands on SBUF partitions, and host-cast to
           bf16 for full-rate matmuls), pT: [64, 2048] fp32 (f32r matmul,
           ~1.5e-4 rel err, so the pos bias factor stays near-fp32).
  loop over 16 row-tiles of 128:
    pos stripe   = PE matmul pT[:, tile].T @ pT          -> PSUM [128, 2048]
                   ScalarE Copy * 1/sqrt(D)              -> SBUF
    per head h:  scores = PE matmul qT[h][:, tile].T @ kT[h] -> PSUM
                 gate   = ScalarE Sigmoid(scores / sqrt(D))  -> SBUF
                 out    = VectorE gate * pos stripe          -> SBUF
                 DMA out stripe (1 MiB, contiguous)          -> HBM
The kernel is output-DMA bound (~50 MiB written per core, ~358 GB/s/core).

Heads 0 and 1 are loaded as one contiguous [128, 2048] SBUF tile (full
16-port DMA bandwidth); head 1's matmul operands therefore live at base
partition 64, which bass lowers to a (64, 0) PE row-tile.
"""

import math
import os
import sys

import numpy as np

sys.path.insert(0, "/opt/trn_rl_repo")

B, H, T, D = 2, 12, 2048, 64
N_CORES = 8
HPC = (B * H) // N_CORES  # heads per core
PT = 128  # output row-tile height (SBUF/PSUM partitions)
NT = T // PT  # row tiles
NCHUNK = 512  # matmul moving-operand free dim (one PSUM bank of fp32)
NCH = T // NCHUNK
INV_SQRT_D = 1.0 / math.sqrt(D)

_NC_CACHE = {}


def _build_nc():
    import concourse.bass as bass
    from concourse import bacc, mybir, tile

    f32 = mybir.dt.float32
    f32r = mybir.dt.float32r
    bf16 = mybir.dt.bfloat16
    Sigmoid = mybir.ActivationFunctionType.Sigmoid
    Copy = mybir.ActivationFunctionType.Copy

    nc = bacc.Bacc("TRN2", target_bir_lowering=False)

    qT = nc.dram_tensor("qT", [HPC, D, T], bf16, kind="ExternalInput")
    kT = nc.dram_tensor("kT", [HPC, D, T], bf16, kind="ExternalInput")
    pT = nc.dram_tensor("pT", [D, T], f32r, kind="ExternalInput")
    out = nc.dram_tensor("out", [HPC, T, T], f32, kind="ExternalOutput")

    with tile.TileContext(nc) as tc:
        with tc.tile_pool(name="ins", bufs=1) as ins_pool, \
             tc.tile_pool(name="pos", bufs=2) as pos_pool, \
             tc.tile_pool(name="gate", bufs=4) as gate_pool, \
             tc.tile_pool(name="outs", bufs=6) as outs_pool, \
             tc.tile_pool(name="ps", bufs=2, space="PSUM") as ps_pool:

            # Load order matters for pipeline ramp-up: the first stripes
            # need pT (pos bias) and head 0's q/k, so load those first.
            # Heads 0+1 are one contiguous [128, 2048] DMA (full port BW).
            p_sb = ins_pool.tile([D, T], f32r, tag="p")
            nc.sync.dma_start(out=p_sb, in_=pT[:])
            k01 = ins_pool.tile([2 * D, T], bf16, tag="k01")
            nc.sync.dma_start(out=k01, in_=kT[0:2].rearrange("h d t -> (h d) t"))
            q01 = ins_pool.tile([2 * D, T], bf16, tag="q01")
            nc.sync.dma_start(out=q01, in_=qT[0:2].rearrange("h d t -> (h d) t"))
            k2 = ins_pool.tile([D, T], bf16, tag="k2")
            nc.sync.dma_start(out=k2, in_=kT[2])
            q2 = ins_pool.tile([D, T], bf16, tag="q2")
            nc.sync.dma_start(out=q2, in_=qT[2])

            q_sb = [q01[0:D, :], q01[D : 2 * D, :], q2]
            k_sb = [k01[0:D, :], k01[D : 2 * D, :], k2]

            for it in range(NT):
                tsl = bass.ts(it, PT)

                pp = ps_pool.tile([PT, T], f32, tag="ps")
                for j in range(NCH):
                    nc.tensor.matmul(
                        pp[:, bass.ts(j, NCHUNK)],
                        p_sb[:, tsl],
                        p_sb[:, bass.ts(j, NCHUNK)],
                        start=True,
                        stop=True,
                    )
                pos_sb = pos_pool.tile([PT, T], f32, tag="pos")
                nc.scalar.activation(pos_sb, pp, Copy, scale=INV_SQRT_D)

                for h in range(HPC):
                    sp = ps_pool.tile([PT, T], f32, tag="ps")
                    for j in range(NCH):
                        nc.tensor.matmul(
                            sp[:, bass.ts(j, NCHUNK)],
                            q_sb[h][:, tsl],
                            k_sb[h][:, bass.ts(j, NCHUNK)],
                            start=True,
                            stop=True,
                        )
                    gate = gate_pool.tile([PT, T], f32, tag="gate")
                    nc.scalar.activation(gate, sp, Sigmoid, scale=INV_SQRT_D)
                    o = outs_pool.tile([PT, T], f32, tag="o")
                    nc.vector.tensor_mul(o, gate, pos_sb)
                    nc.sync.dma_start(out=out[h, tsl, :], in_=o)

    nc.finalize()
    return nc


def _get_nc():
    if "nc" not in _NC_CACHE:
        _NC_CACHE["nc"] = _build_nc()
    return _NC_CACHE["nc"]


def kernel(query, key, pos_embed_weight):
    import ml_dtypes

    query = np.asarray(query, dtype=np.float32)
    key = np.asarray(key, dtype=np.float32)
    pos_embed_weight = np.asarray(pos_embed_weight, dtype=np.float32)

    q = query.reshape(B * H, T, D)
    k = key.reshape(B * H, T, D)
    p_t = np.ascontiguousarray(pos_embed_weight[:T].T)  # [D, T]

    bf = ml_dtypes.bfloat16
    in_maps = []
    for c in range(N_CORES):
        hs = slice(c * HPC, (c + 1) * HPC)
        in_maps.append(
            {
                "qT": np.ascontiguousarray(
                    q[hs].transpose(0, 2, 1).astype(bf)
                ),
                "kT": np.ascontiguousarray(
                    k[hs].transpose(0, 2, 1).astype(bf)
                ),
                "pT": p_t,
            }
        )

    from concourse.bass_utils import run_bass_kernel_spmd

    nc = _get_nc()
    res = run_bass_kernel_spmd(
        nc,
        in_maps,
        core_ids=list(range(N_CORES)),
        trace=bool(os.environ.get("KERNEL_TRACE")),
    )
    kernel.last_results = res

    full = np.empty((B * H, T, T), dtype=np.float32)
    for c in range(N_CORES):
        full[c * HPC : (c + 1) * HPC] = res.results[c]["out"]
    return full.reshape(B, H, T, T)


kernel.last_results = None
